# revision 25
# baseline (speedup 1.0000x reference)
"""
DistancePredictor Trainium2 kernel.

Math:
  xi = x @ Wi + bi            [B, L, H]
  xj = x @ Wj + bj            [B, L, H]
  out = relu(xi[:,:,None,:] * xj[:,None,:,:]) @ Wo + bo    [B, L, L, NB]

Key identity (exact in fp arithmetic, terms have disjoint support):
  relu(a*b) = relu(a)relu(b) + relu(-a)relu(-b)
so
  out[i,j,n] = sum_h (A+[i,h]B+[j,h] + A-[i,h]B-[j,h]) * Wo[h,n] + bo[n]
with A± = relu(±xi), B± = relu(±xj).  This makes the whole
pair/relu/contract pipeline pure TensorE matmuls — no [B,L,L,H]
intermediate ever exists.  Signs are arranged as:
  A+ = max(xi+bi, 0) (DVE),  A- = min(xi+bi, 0)      (= -relu(-(xi+bi)))
  B+ = relu(xj+bj)   (ACT),  B-'= relu(-(xj+bj))     (= -min)
  at+ = A+ * Wo,  at- = A- * (-Wo)   so  at-·B-' = relu(-xi)relu(-xj)·Wo.

Sharding: 8 cores; core c handles batch b=c//4 and i-rows
[96*(c%4), 96*(c%4)+96).  Weights replicated.

Layout tricks:
 - x[b] is transposed AND rolled by -i0 on the host, so the core's own
   96 i-rows are columns 0:96 of its xbt — no separate sliced input.
   The j axis is therefore rolled per-core; undone during host unshard.
 - First layer runs in bf16 (host-cast): halves input DMA bytes; the
   second layer runs fp32r from on-chip fp32 PSUM results.
 - Wi|Wj are packed into one [D, 2H] tensor so each contraction chunk
   is a single DMA per ring (sync ring carries x, scalar ring carries W).
 - xiT is computed directly in [h, i] layout — no on-chip transpose.
 - Output is produced in [NB, 96, L] layout (contiguous DMA per n-pair),
   transposed to [96, L, NB] on the host during unshard.
 - A burst of dummy matmuls on already-landed data right after the first
   chunk keeps TensorE continuously busy so the PE HAM clock un-throttles
   (1.2 -> 2.4 GHz) before the real work peaks.
"""

import numpy as np
import ml_dtypes

import concourse.bass as bass
import concourse.mybir as mybir
import concourse.tile as tile
from concourse import bacc, bass_utils
from concourse.tile_autobufs import add_dep_helper

# Problem constants (hardcoded per contract).
B, L, D, H, NB = 2, 384, 1280, 256, 10
P = 128
KT = D // P     # 10 contraction chunks of 128
KC = 2          # k-chunks per DMA
HT = H // P     # 2 h-chunks of 128
NCORES = 8
IB = (B * L) // NCORES   # 96 i-rows per core
N_WARM = 10     # HAM warm-up dummy matmuls

F32 = mybir.dt.float32
F32R = mybir.dt.float32r
BF16 = mybir.dt.bfloat16
ALU = mybir.AluOpType
RELU = mybir.ActivationFunctionType.Relu

F16 = mybir.dt.float16

# Matmul operand dtypes.  fp16 streams 1 col/cycle on the PE (fp32r does 2)
# and halves input DMA bytes, at ~5e-4 absmax rel err (fp32r: ~2.3e-4).
FIRST_DT = F16   # first-layer operands (x, Wi|Wj) — also sets input DMA width
MAIN_DT = F16    # second-layer operands (b±, at±), produced on-chip
_NP_OF = {F16: np.float16, BF16: ml_dtypes.bfloat16, F32R: np.float32}
_FIRST_NP = _NP_OF[FIRST_DT]

_last_result = None  # BassKernelResults of the most recent run (for test harness)


def build_nc():
    nc = bacc.Bacc("TRN2")

    xbt = nc.dram_tensor("xbt", [D, L], FIRST_DT, kind="ExternalInput")    # roll(x[b].T, -i0)
    wij = nc.dram_tensor("wij", [D, 2 * H], FIRST_DT, kind="ExternalInput")  # [Wi | Wj]
    wo2 = nc.dram_tensor("wo2", [P, 2, HT, NB], F32, kind="ExternalInput")  # [Wo, -Wo]
    biases = nc.dram_tensor("biases", [P, HT, 4], F32, kind="ExternalInput")  # bi,-bi,bj,-bj
    bo_rep = nc.dram_tensor("bo_rep", [P, NB], F32, kind="ExternalInput")     # bo replicated
    # [n-pair, i, n-parity, j]: each pair-DMA writes one fully contiguous block
    out = nc.dram_tensor("out", [NB // 2, IB, 2, L], F32, kind="ExternalOutput")

    xbt_r = xbt[:].rearrange("(c k p) j -> p c k j", p=P, k=KC)   # [128, 5, 2, 384]
    wij_r = wij[:].rearrange("(c k p) h -> p c k h", p=P, k=KC)   # [128, 5, 2, 512]

    with tile.TileContext(nc) as tc:
        with (
            tc.tile_pool(name="persist", bufs=1) as pp,
            tc.tile_pool(name="psA", bufs=2, space="PSUM") as psA_pool,
            tc.tile_pool(name="psB", bufs=2, space="PSUM") as psB_pool,
            tc.tile_pool(name="psO", bufs=4, space="PSUM") as psO_pool,
            tc.tile_pool(name="stage", bufs=3) as stage_pool,
            tc.tile_pool(name="apm", bufs=2) as apm_pool,
        ):
            # ---- persistent SBUF tiles ----
            tl = lambda shape, name, dt=F32: pp.tile(shape, dt, name=name, tag=name)
            xbt_sb = tl([P, KT, L], "xbt_sb", FIRST_DT)
            wij_sb = tl([P, KT, 2 * H], "wij_sb", FIRST_DT)
            wo_sb = tl([P, 2, HT, NB], "wo_sb")
            bias_sb = tl([P, HT, 4], "bias_sb")
            bo_sb = tl([P, NB], "bo_sb")

            bp_sb = tl([P, HT, L], "bp_sb", MAIN_DT)     # relu(xj+bj)      [h, j]
            bm_sb = tl([P, HT, L], "bm_sb", MAIN_DT)     # relu(-(xj+bj))
            atp_sb = tl([P, HT, NB, IB], "atp_sb", MAIN_DT)  # max(xi+bi,0) *  Wo  [h, n, i]
            atm_sb = tl([P, HT, NB, IB], "atm_sb", MAIN_DT)  # min(xi+bi,0) * -Wo

            # ---- small constant DMAs (SWDGE; keeps both HW rings free) ----
            nc.gpsimd.dma_start(wo_sb[:], wo2[:])
            nc.gpsimd.dma_start(bias_sb[:], biases[:])
            nc.gpsimd.dma_start(bo_sb[:], bo_rep[:])

            psA0 = psA_pool.tile([P, IB], F32, name="psA", tag="psA")
            psA1 = psA_pool.tile([P, IB], F32, name="psA", tag="psA")
            psA = [psA0, psA1]
            psB0 = psB_pool.tile([P, L], F32, name="psB", tag="psB")
            psB1 = psB_pool.tile([P, L], F32, name="psB", tag="psB")
            psB = [psB0, psB1]

            # ---- input DMAs: two HWDGE rings in parallel, 2k-sized chunks.
            # Wi chunks are issued before Wj so the whole A-side chain
            # (psA -> a± -> at±) completes while Wj is still streaming; the
            # main loop then starts right after the last B matmul.
            xbt_c = xbt_sb[:].rearrange("p (c k) j -> p c k j", k=KC)
            wij_c = wij_sb[:].rearrange("p (c k) h -> p c k h", k=KC)
            NC_ = KT // KC
            for c in range(NC_):
                nc.sync.dma_start(xbt_c[:, c], xbt_r[:, c])
                nc.scalar.dma_start(wij_c[:, c, :, :H], wij_r[:, c, :, :H])
            # wj: first 3 chunk-groups on the scalar ring, last 2 on sync,
            # so both rings carry ~equal bytes and finish together.
            for c in range(3):
                nc.scalar.dma_start(wij_c[:, c, :, H:], wij_r[:, c, :, H:])
            for c in range(3, NC_):
                nc.sync.dma_start(wij_c[:, c, :, H:], wij_r[:, c, :, H:])

            # ---- first layer: accumulate over k in PSUM.  A-matmuls are
            # gated on (wi, xbt) chunks which arrive first; B-matmuls on wj.
            def junk(n_junk, n_free=P):
                # HAM filler: dummy matmuls on already-landed chunk-0 data
                # keep the PE gapless through DMA waits so the clock gate
                # opens (2.4 GHz) and stays open.
                psW = psO_pool.tile([IB, L], F32, name="psW", tag="psO")
                for w in range(n_junk):
                    nc.tensor.matmul(psW[:, :n_free], wij_sb[:, 0, 0:IB],
                                     xbt_sb[:, 0, :n_free],
                                     start=True, stop=True,
                                     skip_group_check=True)

            last_a = None
            with tc.high_priority():
                for k in range(KT):
                    st, sp = (k == 0), (k == KT - 1)
                    for t in range(HT):
                        # xiT[t][h, i] += wi_k[:, t].T @ x_rows_k   (N=96)
                        last_a = nc.tensor.matmul(psA[t][:],
                                                  wij_sb[:, k, t * P:(t + 1) * P],
                                                  xbt_sb[:, k, :IB],
                                                  start=st, stop=sp)
                    if k == 1:
                        junk(4)
                    elif k in (3, 5, 7):
                        junk(3)
            junk(6, n_free=L)
            for k in range(KT):
                st, sp = (k == 0), (k == KT - 1)
                for t in range(HT):
                    # xjT[t][h, j] += wj_k[:, t].T @ xbt_k      (N=384)
                    bmm = nc.tensor.matmul(psB[t][:],
                                           wij_sb[:, k, H + t * P:H + (t + 1) * P],
                                           xbt_sb[:, k, :],
                                           start=st, stop=sp)
                    if k == 0 and t == 0 and last_a is not None:
                        # keep the whole A accumulation ahead of B in the PE
                        # FIFO: the A-side gates the long DVE at±-chain, while
                        # B only gates the (later) main matmuls.
                        add_dep_helper(bmm.ins, last_a.ins, sync=False,
                                       reason="A-phase before B-phase")


            # ---- A side: a+ = relu(xi+bi) on DVE, a-' = relu(-(xi+bi)) on
            # ACT (parallel engines).  Both are non-negative; the minus-branch
            # sign cancels against b-' = relu(-(xj+bj)).
            ap_ts, am_ts = [], []
            for t in range(HT):
                ap_t = apm_pool.tile([P, IB], F32, name="ap_t", tag=f"ap_t{t}")
                am_t = apm_pool.tile([P, IB], F32, name="am_t", tag=f"am_t{t}")
                nc.vector.tensor_scalar(ap_t[:], psA[t][:],
                                        bias_sb[:, t, 0:1], 0.0,
                                        ALU.add, ALU.max)
                nc.scalar.activation(am_t[:], psA[t][:], RELU,
                                     bias=bias_sb[:, t, 1:2], scale=-1.0)
                ap_ts.append(ap_t)
                am_ts.append(am_t)

            # ---- B side on ACT: bp = relu(xj+bj), bm = relu(-(xj+bj)) ----
            for t in range(HT):
                nc.scalar.activation(bp_sb[:, t, :], psB[t][:], RELU,
                                     bias=bias_sb[:, t, 2:3], scale=1.0)
                nc.scalar.activation(bm_sb[:, t, :], psB[t][:], RELU,
                                     bias=bias_sb[:, t, 3:4], scale=-1.0)

            # at±[h, n, i] = a±[h, i] * (±Wo)[h, n], split by n-halves so the
            # first main matmuls can start early.
            for lo, hi in ((0, 2), (2, 6), (6, 10)):
                ns = slice(lo, hi)
                NH = hi - lo
                for t in range(HT):
                    nc.vector.tensor_tensor(
                        atp_sb[:, t, ns],
                        ap_ts[t][:, None, :].to_broadcast((P, NH, IB)),
                        wo_sb[:, 0, t, ns, None].to_broadcast((P, NH, IB)),
                        ALU.mult)
                    nc.vector.tensor_tensor(
                        atm_sb[:, t, ns],
                        am_ts[t][:, None, :].to_broadcast((P, NH, IB)),
                        wo_sb[:, 0, t, ns, None].to_broadcast((P, NH, IB)),
                        ALU.mult)

            # ---- main contraction: per n, 4 accumulating matmuls ----
            out_pair = out[:]
            for np_ in range(NB // 2):
                ostage = stage_pool.tile([IB, 2, L], F32, name="ostage", tag="ostage")
                for par in range(2):
                    n = np_ * 2 + par
                    psO = psO_pool.tile([IB, L], F32, name="psO", tag="psO")
                    nc.tensor.matmul(psO[:], atp_sb[:, 0, n, :], bp_sb[:, 0, :],
                                     start=True, stop=False)
                    nc.tensor.matmul(psO[:], atp_sb[:, 1, n, :], bp_sb[:, 1, :],
                                     start=False, stop=False)
                    nc.tensor.matmul(psO[:], atm_sb[:, 0, n, :], bm_sb[:, 0, :],
                                     start=False, stop=False)
                    nc.tensor.matmul(psO[:], atm_sb[:, 1, n, :], bm_sb[:, 1, :],
                                     start=False, stop=True)
                    # + bo[n]: alternate engines so neither becomes critical
                    if par == 0:
                        nc.vector.tensor_scalar_add(ostage[:, par, :], psO[:],
                                                    bo_sb[:IB, n:n + 1])
                    else:
                        nc.scalar.activation(
                            ostage[:, par, :], psO[:],
                            mybir.ActivationFunctionType.Identity,
                            bias=bo_sb[:IB, n:n + 1], scale=1.0)
                eng = nc.sync if np_ % 2 == 0 else nc.scalar
                eng.dma_start(out_pair[np_], ostage[:])

    return nc


def _prep_inputs(x, Wi, bi, Wj, bj, Wo, bo):
    """Build the 8 per-core input maps."""
    f = lambda a: np.ascontiguousarray(np.asarray(a, dtype=np.float32))
    x, Wi, bi, Wj, bj, Wo, bo = map(f, (x, Wi, bi, Wj, bj, Wo, bo))

    wij = np.ascontiguousarray(np.hstack([Wi, Wj]).astype(_FIRST_NP))
    wo_r = Wo.reshape(HT, P, NB).transpose(1, 0, 2)                        # [128, 2, 10]
    wo2 = np.ascontiguousarray(np.stack([wo_r, -wo_r], axis=1))            # [128, 2, 2, 10]
    br = lambda v: v.reshape(HT, P).T                                      # [128, 2]
    biases = np.ascontiguousarray(np.stack([br(bi), -br(bi), br(bj), -br(bj)], axis=2))
    bo_rep = np.ascontiguousarray(np.tile(bo[None, :], (P, 1)))            # [128, 10]

    xT = [x[b].T for b in range(B)]                                        # [1280, 384]
    in_maps = []
    for c in range(NCORES):
        b, i0 = c // (NCORES // B), (c % (NCORES // B)) * IB
        in_maps.append({
            "xbt": np.ascontiguousarray(
                np.roll(xT[b], -i0, axis=1).astype(_FIRST_NP)),
            "wij": wij, "wo2": wo2, "biases": biases, "bo_rep": bo_rep,
        })
    return in_maps


def _run(inputs, trace=False):
    global _last_result
    nc = build_nc()
    if not nc.is_finalized():
        nc.finalize()
    in_maps = _prep_inputs(**inputs)
    res = bass_utils.run_bass_kernel_spmd(
        nc, in_maps, core_ids=list(range(NCORES)), trace=trace)
    _last_result = res
    full = np.empty((B, L, L, NB), dtype=np.float32)
    for c in range(NCORES):
        b, i0 = c // (NCORES // B), (c % (NCORES // B)) * IB
        o = res.results[c]["out"]          # [NB/2, IB, 2, L], j rolled by -i0
        o = o.transpose(1, 3, 0, 2).reshape(IB, L, NB)   # -> [i, j_rolled, n]
        full[b, i0:i0 + IB] = np.roll(o, i0, axis=1)
    return full


def kernel(**inputs):
    return _run(inputs, trace=False)
